# revision 1
# baseline (speedup 1.0000x reference)
"""Cross-attention kernel for Trainium2 (8 NeuronCores, SPMD data-parallel).

Problem: O = softmax(Q @ K^T) @ V with B=4, Lq=Lk=4096, D=64, fp32 (no
1/sqrt(d) scaling).

Sharding: 8 cores = 4 batches x 2 Lq-halves. Each core handles a
[2048, 64] Q shard against the full [4096, 64] K/V of its batch.
Independent outputs -> no collectives.

Per-core algorithm (layouts chosen so nothing is transposed on-chip):
  - Host supplies QT [64, 2048] / KT [64, 4096] in fp16 (D on partitions),
    duplicated on-chip across both partition halves so two k-chunks'
    score matmuls run concurrently in the PE array via row tiling
    (contraction is only 64 rows deep).
  - ST[k, q] = matmul(lhsT=KT chunk [64,128], rhs=QT [64,512]) -> PSUM.
  - PT = exp(ST) on the scalar engine, written as bf16 (no max
    subtraction: |scores| < ~50, exp fits fp32/bf16 range; fp16 P would
    underflow).  The scalar engine at 1 elem/cycle/lane is the kernel's
    bottleneck, so exp instructions are kept at 1024 free elements.
  - OT[65, q] += matmul(lhsT=VA chunk [128, 65] bf16, rhs=PT [128, 512]):
    VA = concat([V, ones], 1); rows 0..63 accumulate unnormalized output,
    row 64 the softmax denominator. PT is consumed directly as lhsT-free
    rhs - no transpose anywhere.
  - Normalize: fast-reciprocal of row 64, gpsimd partition-broadcast,
    multiply, DMA out OT [64, 2048]; host transposes back.
"""

import sys

for _p in ("/opt/trn_rl_repo", "/opt/pypackages"):
    if _p not in sys.path:
        sys.path.insert(0, _p)

from contextlib import ExitStack

import ml_dtypes
import numpy as np

import concourse.bacc as bacc
import concourse.mybir as mybir
import concourse.tile as tile
from concourse.bass_utils import run_bass_kernel_spmd

# Problem constants (hardcoded per contract).
B, LQ, LK, D = 4, 4096, 4096, 64
N_CORES = 8
LQ_SHARD = LQ * B // N_CORES  # 2048
QB = 1024  # q-block (exp instruction free-size; 2 PSUM banks)
NQB = LQ_SHARD // QB  # 2
KC = 128  # k-chunk (contraction tile for the PV matmul)
NKC = LK // KC  # 32
SL = 512  # matmul moving-dim slice (one PSUM bank)
NSL = QB // SL  # 2

F32 = mybir.dt.float32
F16 = mybir.dt.float16
BF16 = mybir.dt.bfloat16

BF16NP = ml_dtypes.bfloat16

PACK_S = True  # row-tile two k-chunks' score matmuls concurrently
FAST_RECIP = True  # approx+NR reciprocal (~2 ULP) instead of exact (~6.5us)

KT_PIECE = 512  # kt DMA piece width (cols); 4 k-chunks per piece
VA_PIECE = 8  # va DMA piece size in k-chunks


def _build_program():
    nc = bacc.Bacc(
        "TRN2",
        target_bir_lowering=False,
        debug=False,
        num_devices=N_CORES,
    )
    qt_d = nc.declare_dram_parameter("QT", [D, LQ_SHARD], F16, isOutput=False)
    kt_d = nc.declare_dram_parameter("KT", [D, LK], F16, isOutput=False)
    va_d = nc.declare_dram_parameter("VA", [LK, D + 1], BF16, isOutput=False)
    ot_d = nc.declare_dram_parameter("OT", [D, LQ_SHARD], F32, isOutput=True)

    with tile.TileContext(nc) as tc, ExitStack() as ctx:
        singles = ctx.enter_context(tc.tile_pool(name="singles", bufs=1))
        st_pool = ctx.enter_context(tc.tile_pool(name="st", bufs=2, space="PSUM"))
        ot_pool = ctx.enter_context(tc.tile_pool(name="ot", bufs=2, space="PSUM"))
        pt_pool = ctx.enter_context(tc.tile_pool(name="pt", bufs=3))
        out_pool = ctx.enter_context(tc.tile_pool(name="out", bufs=2))
        norm_pool = ctx.enter_context(tc.tile_pool(name="norm", bufs=4))

        # Preload the exp activation table while input DMAs run.
        warm = singles.tile([1, 2], F32)
        nc.vector.memset(warm[:, :], 0.0)
        nc.scalar.activation(
            out=warm[:, :], in_=warm[:, :],
            func=mybir.ActivationFunctionType.Exp,
        )

        # QT/KT duplicated across both partition halves for PE row tiling.
        # Inputs are split into halves (separate tiles) so the first score
        # matmuls don't wait for the full 2 MB of loads; keeping the piece
        # count low preserves the Tile scheduler's pairing of the row-tiled
        # matmuls (many small tiles reorder the PE stream and let HAM
        # re-throttle the PE clock).
        va_r = va_d[:, :].rearrange("(c p) d -> p c d", p=KC)
        KH = LK // 2  # kt half width
        VH = NKC // 2  # va half size in chunks
        kt_sb = []
        qt_sb = []
        va_sb = []
        for h in range(2):
            tq = singles.tile([2 * D, QB], F16, name=f"qt{h}")
            sq = slice(h * QB, (h + 1) * QB)
            nc.sync.dma_start(out=tq[0:D, :], in_=qt_d[:, sq])
            nc.sync.dma_start(out=tq[D : 2 * D, :], in_=qt_d[:, sq])
            qt_sb.append(tq)
            t = singles.tile([2 * D, KH], F16, name=f"kt{h}")
            sl = slice(h * KH, (h + 1) * KH)
            nc.sync.dma_start(out=t[0:D, :], in_=kt_d[:, sl])
            nc.sync.dma_start(out=t[D : 2 * D, :], in_=kt_d[:, sl])
            kt_sb.append(t)
            tv = singles.tile([KC, VH, D + 1], BF16, name=f"va{h}")
            nc.sync.dma_start(
                out=tv[:, :, :], in_=va_r[:, h * VH : (h + 1) * VH, :]
            )
            va_sb.append(tv)

        def kt_ap(half, c):
            # [64, 128] fp16 weights for chunk c from partition half `half`
            t = kt_sb[c * KC // KH]
            off = (c * KC) % KH
            return t[half * D : (half + 1) * D, off : off + KC]

        def va_ap(c):
            return va_sb[c // VH][:, c % VH, :]

        for qb in range(NQB):
            ot_ps = ot_pool.tile([D + 1, QB], F32)
            for cp in range(NKC // 2):  # chunk pairs, row-tiled in the PE
                c0, c1 = 2 * cp, 2 * cp + 1
                st_a = st_pool.tile([KC, QB], F32, tag="st")
                st_b = st_pool.tile([KC, QB], F32, tag="st")
                for s in range(NSL):
                    q0 = qb * QB + s * SL
                    qt = qt_sb[qb]
                    if PACK_S:
                        nc.tensor.matmul(
                            out=st_a[:, s * SL : (s + 1) * SL],
                            lhsT=kt_ap(0, c0),
                            rhs=qt[0:D, s * SL : (s + 1) * SL],
                            start=True,
                            stop=True,
                            tile_position=(0, 0),
                        )
                        nc.tensor.matmul(
                            out=st_b[:, s * SL : (s + 1) * SL],
                            lhsT=kt_ap(1, c1),
                            rhs=qt[D : 2 * D, s * SL : (s + 1) * SL],
                            start=True,
                            stop=True,
                            tile_position=(D, 0),
                        )
                    else:
                        nc.tensor.matmul(
                            out=st_a[:, s * SL : (s + 1) * SL],
                            lhsT=kt_ap(0, c0),
                            rhs=qt[0:D, s * SL : (s + 1) * SL],
                            start=True,
                            stop=True,
                        )
                        nc.tensor.matmul(
                            out=st_b[:, s * SL : (s + 1) * SL],
                            lhsT=kt_ap(0, c1),
                            rhs=qt[0:D, s * SL : (s + 1) * SL],
                            start=True,
                            stop=True,
                        )
                for c, st_ps in ((c0, st_a), (c1, st_b)):
                    pt = pt_pool.tile([KC, QB], BF16)
                    nc.scalar.activation(
                        out=pt[:, :],
                        in_=st_ps[:, :],
                        func=mybir.ActivationFunctionType.Exp,
                    )
                    for s in range(NSL):
                        nc.tensor.matmul(
                            out=ot_ps[:, s * SL : (s + 1) * SL],
                            lhsT=va_ap(c),
                            rhs=pt[:, s * SL : (s + 1) * SL],
                            start=(c == 0),
                            stop=(c == NKC - 1),
                        )
            # Normalize: O[d, q] = OT[d, q] / OT[64, q]
            recip = norm_pool.tile([1, QB], F32)
            if FAST_RECIP:
                den = norm_pool.tile([1, QB], F32)
                nc.vector.tensor_copy(den[:, :], ot_ps[D : D + 1, :])
                scratch = norm_pool.tile([1, QB], F32)
                nc.vector.reciprocal_approx_accurate(
                    recip[:, :], den[:, :], scratch[:, :]
                )
            else:
                nc.vector.reciprocal(out=recip[:, :], in_=ot_ps[D : D + 1, :])
            bcast = norm_pool.tile([D, QB], F32)
            nc.gpsimd.partition_broadcast(bcast[:, :], recip[:, :])
            o_sb = out_pool.tile([D, QB], F32)
            nc.vector.tensor_mul(o_sb[:, :], ot_ps[0:D, :], bcast[:, :])
            nc.sync.dma_start(
                out=ot_d[:, qb * QB : (qb + 1) * QB], in_=o_sb[:, :]
            )

    nc.finalize()
    return nc


_PROGRAM_CACHE = {}


def _get_program():
    if "nc" not in _PROGRAM_CACHE:
        _PROGRAM_CACHE["nc"] = _build_program()
    return _PROGRAM_CACHE["nc"]


def _make_in_maps(Q, K, V):
    Q = np.asarray(Q, dtype=np.float32)
    K = np.asarray(K, dtype=np.float32)
    V = np.asarray(V, dtype=np.float32)
    in_maps = []
    ones = np.ones((LK, 1), dtype=np.float32)
    for core in range(N_CORES):
        b, half = core // 2, core % 2
        q_shard = Q[b, half * LQ_SHARD : (half + 1) * LQ_SHARD, :]  # [2048, 64]
        qt = np.ascontiguousarray(q_shard.T).astype(np.float16)  # [64, 2048]
        kt = np.ascontiguousarray(K[b].T).astype(np.float16)  # [64, 4096]
        va = np.concatenate([V[b], ones], axis=1).astype(BF16NP)  # [4096, 65]
        in_maps.append({"QT": qt, "KT": kt, "VA": np.ascontiguousarray(va)})
    return in_maps


def _run(Q, K, V, trace=False, **spmd_kwargs):
    nc = _get_program()
    in_maps = _make_in_maps(Q, K, V)
    res = run_bass_kernel_spmd(
        nc, in_maps, list(range(N_CORES)), trace=trace, **spmd_kwargs
    )
    out = np.empty((B, LQ, D), dtype=np.float32)
    for core in range(N_CORES):
        b, half = core // 2, core % 2
        ot = res.results[core]["OT"]  # [64, 2048]
        out[b, half * LQ_SHARD : (half + 1) * LQ_SHARD, :] = ot.T
    return out, res


def kernel(Q, K, V):
    out, _ = _run(Q, K, V, trace=False)
    return out



# revision 11
# speedup vs baseline: 1.1052x; 1.1052x over previous
"""Cross-attention kernel for Trainium2 (8 NeuronCores, SPMD data-parallel).

Problem: O = softmax(Q @ K^T) @ V with B=4, Lq=Lk=4096, D=64, fp32 (no
1/sqrt(d) scaling).

Sharding: 8 cores = 4 batches x 2 Lq-halves. Each core handles a
[2048, 64] Q shard against the full [4096, 64] K/V of its batch.
Independent outputs -> no collectives.

Per-core algorithm (v2 — dual-engine exp):
  - Host supplies QT [64, 2048] fp16 PRE-SCALED by 1/256 (exact: power of
    two), KT [64, 4096] fp16; both duplicated across partition halves so
    two k-chunks' score matmuls run row-tiled concurrently in the PE.
  - ST[k, q] = matmul(...) -> PSUM holds s/256.
  - exp is split across BOTH the Scalar and Vector engines (the baseline
    was scalar-bound at ~72us busy):
      * Scalar path: activation(Exp, scale=256) -> bf16 PT.
      * Vector path: two 8-stage custom DVE ops registered at runtime:
        p = poly4(t) ~ e^t (minimax, c0=c1=1, rel err 1.05e-6 on
        |t|<=0.25), then p^256 via 8 chained squarings -> bf16 PT.
        (Src1 past stage 5 crashes the DVE; the c1=1 constraint avoids it.)
    Chunks are assigned greedily by modeled per-tile cost (scalar
    ~1.05us, DVE ~2.46us for a [128,1024] tile).
  - OT[65, q] += matmul(lhsT=VA chunk [128, 65] bf16, rhs=PT [128, 512]);
    VA = concat([V, ones], 1); row 64 accumulates the softmax denominator.
    PV matmuls are emitted in predicted-exp-completion order, interleaved
    one pair behind the score matmuls, to keep the PE stream dense.
  - Normalize: DVE fast-reciprocal of row 64, then gpsimd broadcast +
    multiply (gpsimd is otherwise idle; DVE/scalar are exp-critical),
    in 2 pieces of 512 columns to pipeline the tail; DMA out.
"""

import sys

for _p in ("/opt/trn_rl_repo", "/opt/pypackages"):
    if _p not in sys.path:
        sys.path.insert(0, _p)

from contextlib import ExitStack

import ml_dtypes
import numpy as np

import concourse.bacc as bacc
import concourse.mybir as mybir
import concourse.tile as tile
from concourse.bass_utils import run_bass_kernel_spmd

import concourse.dve_ops as _dve_ops
from concourse.dve_spec import C0, C1, C2, One, Spec, Src0, lower
from concourse.dve_spec import _has_src1
from concourse.dve_uop import DveOpSpec

# ---------------------------------------------------------------- constants
B, LQ, LK, D = 4, 4096, 4096, 64
N_CORES = 8
LQ_SHARD = LQ * B // N_CORES  # 2048
QB = 1024  # q-block (exp instruction free-size; 2 PSUM banks)
NQB = LQ_SHARD // QB  # 2
KC = 128  # k-chunk (contraction tile for the PV matmul)
NKC = LK // KC  # 32
SL = 512  # matmul moving-dim slice (one PSUM bank)
NSL = QB // SL  # 2
SCORE_SCALE = 256.0  # host pre-scales Q by 1/256 (exact)
VA_W = 128  # VA columns: [ones | 63 zero pad | V]; den lands on OT row 0
V_OFF = 64  # V starts at column 64 (64-wide accesses must start at 0/64)

F32 = mybir.dt.float32
F16 = mybir.dt.float16
BF16 = mybir.dt.bfloat16
BF16NP = ml_dtypes.bfloat16

# Modeled per-[128,1024]-tile exp costs (us) for the greedy split.
TS = 1.05  # scalar: 1024/1.2GHz + access + seq
TD = 2.46  # DVE: two 1x passes + access + seq
TD_HANDICAP = 2.8  # norm work (recip+mul) of the previous q-block on DVE

# Minimax (c2, c3, c4) for e^t, t in [-0.25, 0.25], c0=c1=1 fixed.
EXP_C2 = 0.5000139854903264
EXP_C3 = 0.16711872930830435
EXP_C4 = 0.04146165926052129

# ------------------------------------------------- custom DVE exp ops
_t = Src0
_POLY_BODY = (((_t * C2 + C1) * _t + C0) * _t + One) * _t + One


def _poly_ref(in0, in1, s0, s1, imm2):
    x = in0.astype(np.float32)
    return ((((x * imm2 + s1) * x + s0) * x + 1.0) * x + 1.0).astype(np.float32)


_x = Src0
for _ in range(8):
    _x = _x * _x


def _sq_ref(in0, in1, s0, s1, imm2):
    return (in0.astype(np.float64) ** 256).astype(np.float32)


def _register_exp_ops():
    existing = {op.name: op for op in _dve_ops.OPS}
    if "EXP_POLY_V2_ANT" in existing:
        return existing["EXP_POLY_V2_ANT"], existing["EXP_SQUARE8_ANT"]
    specs = {
        "EXP_POLY_V2_ANT": Spec(body=_POLY_BODY, reference=_poly_ref),
        "EXP_SQUARE8_ANT": Spec(body=_x, reference=_sq_ref),
    }
    ops = []
    for name, spec in specs.items():
        row = max(_dve_ops._SUB_OPCODE_FOR_NAME.values()) + 1
        assert row < 0x20, "opcode row field overflow"
        _dve_ops._SUB_OPCODE_FOR_NAME[name] = row
        shas = {}
        for ver in ("v3", "v4"):
            try:
                tmp = DveOpSpec(
                    name=name,
                    opcode=row,
                    uops=lower(spec, ver=ver),
                    rd1_en=_has_src1(spec),
                )
                shas[ver] = tmp.sha(ver)
            except Exception:
                pass
        op = _dve_ops.DveOp(name, spec, subdim=False, uops_sha=shas)
        _dve_ops.OPS.append(op)
        _dve_ops.CUSTOM_DVE_SPECS[name] = spec
        ops.append(op)
    return ops[0], ops[1]


def _assign_engines():
    """Greedy chunk->engine split by modeled cost; returns list of 'S'/'D'."""
    import os

    ov = os.environ.get("KERNEL_ASSIGN", "")
    if ov == "ALL_S":
        return ["S"] * NKC
    if ov == "ALL_D":
        return ["D"] * NKC
    t_s, t_d = 0.0, TD_HANDICAP
    out = []
    for _c in range(NKC):
        if t_s + TS <= t_d + TD:
            out.append("S")
            t_s += TS
        else:
            out.append("D")
            t_d += TD
    # std PSUM pool has a single buffer: never put both pair members on DVE.
    for p in range(NKC // 2):
        if out[2 * p] == "D" and out[2 * p + 1] == "D":
            out[2 * p + 1] = "S"
    return out


def _pv_order(assign):
    """Chunk indices sorted by predicted exp completion time."""
    done = {}
    t_s, t_d = 0.0, TD_HANDICAP
    for c, eng in enumerate(assign):
        if eng == "S":
            t_s += TS
            done[c] = t_s
        else:
            t_d += TD
            done[c] = t_d
    return sorted(range(NKC), key=lambda c: (done[c], c))


# ---------------------------------------------------------------- program
def _build_program():
    poly_op, sq_op = _register_exp_ops()

    nc = bacc.Bacc(
        "TRN2",
        target_bir_lowering=False,
        debug=False,
        num_devices=N_CORES,
    )
    qt_d = nc.declare_dram_parameter("QT", [D, LQ_SHARD], F16, isOutput=False)
    kt_d = nc.declare_dram_parameter("KT", [D, LK], F16, isOutput=False)
    va_d = nc.declare_dram_parameter("VA", [LK, VA_W], BF16, isOutput=False)
    ot_d = nc.declare_dram_parameter("OT", [D, LQ_SHARD], F32, isOutput=True)

    assign = _assign_engines()
    pv_order = _pv_order(assign)

    with tile.TileContext(nc) as tc, ExitStack() as ctx:
        singles = ctx.enter_context(tc.tile_pool(name="singles", bufs=1))
        sts_pool = ctx.enter_context(
            tc.tile_pool(name="sts", bufs=2, space="PSUM")
        )
        std_pool = ctx.enter_context(
            tc.tile_pool(name="std", bufs=1, space="PSUM")
        )
        ot_pool = ctx.enter_context(tc.tile_pool(name="ot", bufs=1, space="PSUM"))
        pt_pool = ctx.enter_context(tc.tile_pool(name="pt", bufs=6))
        scr_pool = ctx.enter_context(tc.tile_pool(name="scr", bufs=2))
        out_pool = ctx.enter_context(tc.tile_pool(name="out", bufs=6))
        norm_pool = ctx.enter_context(tc.tile_pool(name="norm", bufs=4))

        # Preload the exp activation table while input DMAs run.
        warm = singles.tile([1, 2], F32)
        nc.vector.memset(warm[:, :], 0.0)
        nc.scalar.activation(
            out=warm[:, :], in_=warm[:, :],
            func=mybir.ActivationFunctionType.Exp,
        )

        # QT/KT duplicated across both partition halves for PE row tiling.
        va_r = va_d[:, :].rearrange("(c p) d -> p c d", p=KC)
        KH = LK // 2
        VH = NKC // 2
        kt_sb = []
        qt_sb = []
        va_sb = []
        for h in range(2):
            tq = singles.tile([2 * D, QB], F16, name=f"qt{h}")
            sq_ = slice(h * QB, (h + 1) * QB)
            nc.sync.dma_start(out=tq[0:D, :], in_=qt_d[:, sq_])
            nc.sync.dma_start(out=tq[D : 2 * D, :], in_=qt_d[:, sq_])
            qt_sb.append(tq)
            t = singles.tile([2 * D, KH], F16, name=f"kt{h}")
            sl = slice(h * KH, (h + 1) * KH)
            nc.sync.dma_start(out=t[0:D, :], in_=kt_d[:, sl])
            nc.sync.dma_start(out=t[D : 2 * D, :], in_=kt_d[:, sl])
            kt_sb.append(t)
            tv = singles.tile([KC, VH, VA_W], BF16, name=f"va{h}")
            nc.sync.dma_start(
                out=tv[:, :, :], in_=va_r[:, h * VH : (h + 1) * VH, :]
            )
            va_sb.append(tv)

        def kt_ap(half, c):
            t = kt_sb[c * KC // KH]
            off = (c * KC) % KH
            return t[half * D : (half + 1) * D, off : off + KC]

        def va_ap(c):
            return va_sb[c // VH][:, c % VH, :]

        for qb in range(NQB):
            ot_ps = ot_pool.tile([VA_W, QB], F32)
            qt = qt_sb[qb]
            st_tiles = {}
            pt_tiles = {}
            pv_emitted = 0

            def emit_scores(p):
                c0, c1 = 2 * p, 2 * p + 1
                pool0 = sts_pool if assign[c0] == "S" else std_pool
                pool1 = sts_pool if assign[c1] == "S" else std_pool
                st_a = pool0.tile([KC, QB], F32, tag="st")
                st_b = pool1.tile([KC, QB], F32, tag="st")
                st_tiles[c0] = st_a
                st_tiles[c1] = st_b
                for s in range(NSL):
                    sl = slice(s * SL, (s + 1) * SL)
                    nc.tensor.matmul(
                        out=st_a[:, sl],
                        lhsT=kt_ap(0, c0),
                        rhs=qt[0:D, sl],
                        start=True,
                        stop=True,
                        tile_position=(0, 0),
                    )
                    nc.tensor.matmul(
                        out=st_b[:, sl],
                        lhsT=kt_ap(1, c1),
                        rhs=qt[D : 2 * D, sl],
                        start=True,
                        stop=True,
                        tile_position=(D, 0),
                    )

            def emit_exps(p):
                for c in (2 * p, 2 * p + 1):
                    st = st_tiles[c]
                    pt = pt_pool.tile([KC, QB], BF16)
                    pt_tiles[c] = pt
                    if assign[c] == "S":
                        nc.scalar.activation(
                            out=pt[:, :],
                            in_=st[:, :],
                            func=mybir.ActivationFunctionType.Exp,
                            scale=SCORE_SCALE,
                        )
                    else:
                        scr = scr_pool.tile([KC, QB], F32)
                        nc.vector._custom_dve(
                            poly_op,
                            out=scr[:, :],
                            in0=st[:, :],
                            s0=EXP_C2,
                            s1=EXP_C3,
                            imm2=EXP_C4,
                        )
                        nc.vector._custom_dve(sq_op, out=pt[:, :], in0=scr[:, :])

            def emit_pvs(max_pair_done, count):
                nonlocal pv_emitted
                n = 0
                while pv_emitted < NKC and n < count:
                    c = pv_order[pv_emitted]
                    if c >= 2 * max_pair_done:
                        break  # its scores are not emitted yet
                    pt = pt_tiles[c]
                    for s in range(NSL):
                        sl = slice(s * SL, (s + 1) * SL)
                        nc.tensor.matmul(
                            out=ot_ps[:, sl],
                            lhsT=va_ap(c),
                            rhs=pt[:, sl],
                            start=(pv_emitted == 0),
                            stop=(pv_emitted == NKC - 1),
                        )
                    pv_emitted += 1
                    n += 1

            # software pipeline: scores run one pair ahead of PVs
            emit_scores(0)
            emit_exps(0)
            emit_scores(1)
            emit_exps(1)
            for p in range(2, NKC // 2):
                emit_pvs(p, 2)
                emit_scores(p)
                emit_exps(p)
            emit_pvs(NKC // 2, NKC)  # drain

            # Normalize in 2 pieces of 512 columns to pipeline the tail.
            # gpsimd/DMA cannot read PSUM, so the PSUM-touching ops (recip,
            # multiply) run on DVE; gpsimd only broadcasts the SBUF recip.
            # VA has the ones-column FIRST, so the denominator is OT row 0:
            # custom DVE ops misread nonzero partition offsets, so the recip
            # input must sit at partition 0.
            for piece in range(NSL):
                sl = slice(piece * SL, (piece + 1) * SL)
                recip = norm_pool.tile([1, SL], F32)
                nc.vector.reciprocal_approx_fast(recip[:, :], ot_ps[0:1, sl])
                bcast = norm_pool.tile([D, SL], F32)
                nc.gpsimd.partition_broadcast(bcast[:, :], recip[:, :])
                o_sb = out_pool.tile([D, SL], F32)
                nc.vector.tensor_mul(
                    o_sb[:, :], ot_ps[V_OFF : V_OFF + D, sl], bcast[:, :]
                )
                nc.sync.dma_start(
                    out=ot_d[:, qb * QB + piece * SL : qb * QB + (piece + 1) * SL],
                    in_=o_sb[:, :],
                )

    nc.finalize()
    return nc


_PROGRAM_CACHE = {}


def _get_program():
    if "nc" not in _PROGRAM_CACHE:
        _PROGRAM_CACHE["nc"] = _build_program()
    return _PROGRAM_CACHE["nc"]


def _make_in_maps(Q, K, V):
    Q = np.asarray(Q, dtype=np.float32)
    K = np.asarray(K, dtype=np.float32)
    V = np.asarray(V, dtype=np.float32)
    in_maps = []
    ones = np.ones((LK, 1), dtype=np.float32)
    for core in range(N_CORES):
        b, half = core // 2, core % 2
        q_shard = Q[b, half * LQ_SHARD : (half + 1) * LQ_SHARD, :]
        qt = np.ascontiguousarray(q_shard.T / SCORE_SCALE).astype(np.float16)
        kt = np.ascontiguousarray(K[b].T).astype(np.float16)
        pad = np.zeros((LK, V_OFF - 1), dtype=np.float32)
        va = np.concatenate([ones, pad, V[b]], axis=1).astype(BF16NP)
        in_maps.append({"QT": qt, "KT": kt, "VA": np.ascontiguousarray(va)})
    return in_maps


def _run(Q, K, V, trace=False, **spmd_kwargs):
    nc = _get_program()
    in_maps = _make_in_maps(Q, K, V)
    res = run_bass_kernel_spmd(
        nc, in_maps, list(range(N_CORES)), trace=trace, **spmd_kwargs
    )
    out = np.empty((B, LQ, D), dtype=np.float32)
    for core in range(N_CORES):
        b, half = core // 2, core % 2
        ot = res.results[core]["OT"]  # [64, 2048]
        out[b, half * LQ_SHARD : (half + 1) * LQ_SHARD, :] = ot.T
    return out, res


def kernel(Q, K, V):
    out, _ = _run(Q, K, V, trace=False)
    return out
